# revision 19
# baseline (speedup 1.0000x reference)
"""Trainium2 Bass kernel for nn_GeneralizedKernelScore (loss_fn).

Math per sample n (M=8 population members, D=12288 features):
    beta      = 2.0 - 1.9*t/999                      (linear schedule from t)
    conf[n]   = mean_j    exp(-beta*||x_j - y_j||^2 / D)
    inter[n]  = mean_{j!=j'} exp(-beta*||x_j - x_j'||^2 / D)
    im[n]     = inter/2
    score[n]  = im - conf

Strategy (data-parallel over batch, 4 samples per core on 8 cores):
Each core owns Z = [X; Y] (64 rows x 12288) in fp8-e4m3, pre-transposed
on the host to feature-major [128, 96*64] so the contraction dim lands
on SBUF partitions.  All distances come from the Gram matrix G = Z Z^T.
Feature chunks are processed two at a time: one matmul per pair with
lhsT = rhs = [chunk_j | chunk_j+1] ([128, 128]) accumulates
    P[0:64, 0:64]     += chunk_j   Gram contribution
    P[64:128, 64:128] += chunk_j+1 Gram contribution
(off-diagonal blocks are cross-chunk junk, ignored).  The 128-column
fp8 weight loads ride the fast-weight-load path and hide behind the
128-cycle streams; a short warm-up spin of junk matmuls starts the PE
early so the HAM clock gate reaches 2.4 GHz while the input still
streams in.

Epilogue (3 cross-engine hops):
  DVE   : xn2 = diag(G) via a stride-129 access pattern; one fused
          tensor_scalar builds the norm-routing rhs and the fold
          weights; one combined mask (same-sample block + x.y diag,
          disjoint) + grouped reduce compacts the -2G terms, with the
          x.y term landing in the f = p%8 slot
  PE    : three matmuls accumulate pt[32,8] = D*d2 args; the diag slot
          becomes the confinement arg, the [128->32] fold of the split
          Gram halves rides the contraction
  DVE   : extract the diag slot (conf arg) before the exp
  ACT   : two Exps with per-partition scale -beta/D (host-computed
          from t): pairs+conf row-sums via accum_out -> sc[:,0],
          conf -> sc[:,1]; the result DMA issues from this same
          engine's HWDGE queue (no extra hop)
  Host  : sums 8 rows per sample and applies the constant affine.

DMA: input split in 4 chunks (small first chunk for an early start)
issued alternately on the two HWDGE queues (SP + Activation) so the
rings drain in parallel; constants ride a 5th transfer.
"""

from contextlib import ExitStack

import numpy as np
import ml_dtypes

import concourse.bass as bass
from concourse.bass_types import AP
import concourse.mybir as mybir
import concourse.tile as tile
from concourse import bacc
from concourse.bass_utils import run_bass_kernel_spmd

# problem shape (hardcoded per spec)
N, M, D = 32, 8, 12288
NUM_TIMESTEPS = 1000
BETA_START, BETA_END = 2.0, 0.1
LAMBDA_VAL = 1.0

NCORES = 8
NS = N // NCORES          # 4 samples per core
R = 2 * NS * M            # 64 Z-rows per core (32 x-rows then 32 y-rows)
NCH = D // 128            # 96 contraction chunks of the feature dim
FREE = NCH * R            # 6144 free columns of Z^T
# input DMA chunk widths (bytes per partition line); must sum to FREE
# and stay multiples of 128 (one ldw-pair)
CHUNKS = [256, 768, 2560, 2560]
N_WARM = 7                # PE warm-up matmuls (N=256 each, gapless to gram)
DIAG_AP = False           # stride-129 diag AP (rejected by birverifier)

# const tensor column layout
_M2C, _I64, _MK8, _W3, _W2, _ON8, _MD, _BV, _P4 = (
    0, 128, 256, 264, 296, 328, 336, 344, 345,
)
CONW = 349

F32 = mybir.dt.float32
FP8 = mybir.dt.float8e4
NP_FP8 = ml_dtypes.float8_e4m3


def _build_consts():
    k = np.arange(128)[:, None]
    km = k % 64
    c = np.arange(128)[None, :]
    xrow = km < 32
    # combined -2 mask: same-sample x-x block (incl diag) + x.y diag;
    # disjoint regions, both land compatibly under the g=16 grouped sum
    m2c = np.where(
        (xrow & (c // 8 == k // 8) & (c % 64 < 32)) | (xrow & (c == k + 32)),
        -2.0, 0.0,
    )
    i64 = (c == k).astype(np.float32)  # fallback diag mask
    f8 = np.arange(8)[None, :]
    mk8 = (k % 8 == f8).astype(np.float32)       # norm routing by j = k%8
    m32 = np.arange(32)[None, :]
    w3 = (xrow & (km == m32)).astype(np.float32)  # fold [128]->[32], x-rows
    # W2 = A (same-sample x-rows) + B (own y-row) + C (own x-row);
    # arithmetic sum: A and C overlap on the own row, weight 2 there
    w2 = (
        (xrow & (km // 8 == m32 // 8)).astype(np.float32)
        + (km == 32 + m32).astype(np.float32)
        + (km == m32).astype(np.float32)
    )
    on8 = np.ones((128, 8), dtype=np.float32)
    md = (xrow & (k % 8 == f8)).astype(np.float32)[: 128]  # diag-slot mask
    bv = np.zeros((128, 1), dtype=np.float32)  # filled per-core with -beta/D
    p4 = ((k < 32) & (k // 8 == np.arange(4)[None, :])).astype(np.float32)
    con = np.concatenate(
        [m2c, i64, mk8, w3, w2, on8, md, bv, p4], axis=1
    ).astype(np.float32)
    assert con.shape == (128, CONW)
    return con


def _build_program():
    nc = bacc.Bacc("TRN2", target_bir_lowering=False)
    zt = nc.dram_tensor("zt", [128, FREE], FP8, kind="ExternalInput")
    con_d = nc.dram_tensor("con", [128, CONW], F32, kind="ExternalInput")
    res_d = nc.dram_tensor("res", [NS, 2], F32, kind="ExternalOutput")

    mult = mybir.AluOpType.mult
    EXP = mybir.ActivationFunctionType.Exp

    with ExitStack() as ctx:
        tc = ctx.enter_context(tile.TileContext(nc))
        small = ctx.enter_context(tc.tile_pool(name="small", bufs=1))
        zbf_p = ctx.enter_context(tc.tile_pool(name="zbf", bufs=len(CHUNKS)))
        psum = ctx.enter_context(tc.tile_pool(name="psum", bufs=1, space="PSUM"))

        # --- PE warm-up spin: open the HAM clock gate early -----------
        wt = small.tile([128, 256], FP8, tag="wt")
        nc.vector.memset(wt, 0.0)
        wp = psum.tile([128, 256], F32, tag="wp")
        for _ in range(N_WARM):
            nc.tensor.matmul(
                wp, lhsT=wt[:, 0:128], rhs=wt, start=True, stop=True,
                skip_group_check=True,
            )

        # --- input + const DMAs, alternating the two HWDGE queues -----
        zbf = []
        off = 0
        for i, w in enumerate(CHUNKS):
            zc = zbf_p.tile([128, w], FP8, tag="zbf")
            eng = nc.sync if i % 2 == 0 else nc.scalar
            eng.dma_start(out=zc, in_=zt[:, off : off + w])
            zbf.append(zc)
            off += w
        con = small.tile([128, CONW], F32, tag="con")
        nc.sync.dma_start(out=con, in_=con_d[:])

        # preload the Exp LUT while DMAs run
        warm = small.tile([1, 1], F32, tag="warm")
        nc.vector.memset(warm, 0.0)
        nc.scalar.activation(out=warm, in_=warm, func=EXP)

        # --- Gram: one [128,128] matmul per chunk pair ----------------
        G = psum.tile([128, 128], F32, tag="G")
        npair = NCH // 2
        p = 0
        for i, w in enumerate(CHUNKS):
            for j in range(w // 128):
                pair = zbf[i][:, j * 128 : (j + 1) * 128]
                nc.tensor.matmul(
                    G, lhsT=pair, rhs=pair,
                    start=(p == 0), stop=(p == npair - 1),
                )
                p += 1
        assert p == npair

        # --- epilogue ---------------------------------------------------
        # [V] xn2 = diag(G): split norms (even-chunk half on rows 0-63,
        # odd on 64-127)
        xn2 = small.tile([128, 1], F32, tag="xn2")
        if DIAG_AP:
            gdiag = AP(tensor=G.tensor, offset=G.offset, ap=[[129, 128], [1, 1]])
            nc.vector.tensor_copy(out=xn2, in_=gdiag)
        else:
            s128 = small.tile([128, 128], F32, tag="s128")
            nc.vector.tensor_tensor(
                out=s128, in0=G, in1=con[:, _I64 : _I64 + 128], op=mult
            )
            nc.vector.reduce_sum(out=xn2, in_=s128, axis=mybir.AxisListType.X)
        # rw = [mask8 | W3] . xn2 : rhs8 = rw[:,0:8], xnw = rw[:,8:40]
        rw = small.tile([128, 40], F32, tag="rw")
        nc.vector.tensor_scalar(
            out=rw, in0=con[:, _MK8 : _MK8 + 40], scalar1=xn2, scalar2=None,
            op0=mult,
        )
        gm = small.tile([128, 128], F32, tag="gm")
        nc.vector.tensor_tensor(
            out=gm, in0=G, in1=con[:, _M2C : _M2C + 128], op=mult
        )
        cmc = small.tile([128, 8], F32, tag="cmc")
        nc.vector.reduce_sum(
            out=cmc,
            in_=gm.rearrange("p (g f) -> p f g", g=16),
            axis=mybir.AxisListType.X,
        )

        # [T] pt[:,0:8] = norm-spread + row-norm bcast + (-2G, folded)
        pt = psum.tile([32, 9], F32, tag="pt")
        nc.tensor.matmul(
            pt[:, 0:8], lhsT=con[:, _W2 : _W2 + 32], rhs=rw[:, 0:8],
            start=True, stop=False,
        )
        nc.tensor.matmul(
            pt[:, 0:8], lhsT=rw[:, 8:40], rhs=con[:, _ON8 : _ON8 + 8],
            start=False, stop=False,
        )
        nc.tensor.matmul(
            pt[:, 0:8], lhsT=con[:, _W3 : _W3 + 32], rhs=cmc,
            start=False, stop=True,
        )

        # [V] copy the confinement arg (diag slot) into pt col 8
        md = small.tile([32, 8], F32, tag="md")
        nc.vector.tensor_tensor(
            out=md, in0=pt[:, 0:8], in1=con[0:32, _MD : _MD + 8], op=mult
        )
        nc.vector.reduce_sum(
            out=pt[:, 8:9], in_=md, axis=mybir.AxisListType.X
        )

        # [S] one exp over [32,9]: w[:,0:9] = exp values (col 8 = conf),
        # accum lands in w[:,9] -> adjacent [conf|rowsum] pair
        w = small.tile([32, 10], F32, tag="w")
        nc.scalar.activation(
            out=w[:, 0:9], in_=pt, func=EXP, scale=con[0:32, _BV : _BV + 1],
            accum_out=w[:, 9:10],
        )

        # [T] per-sample sums over the 8 population rows
        pc = psum.tile([NS, 2], F32, tag="pc")
        nc.tensor.matmul(
            pc, lhsT=con[0:32, _P4 : _P4 + 4], rhs=w[:, 8:10],
            start=True, stop=True,
        )

        # [V] -> DMA out
        fin = small.tile([NS, 2], F32, tag="fin")
        nc.vector.tensor_copy(out=fin, in_=pc)
        nc.sync.dma_start(out=res_d[:], in_=fin)

    nc.compile()
    return nc


_PROG = None
_CONSTS = None


def _get_prog():
    global _PROG
    if _PROG is None:
        _PROG = _build_program()
    return _PROG


def _make_in_maps(x, y, t):
    global _CONSTS
    if _CONSTS is None:
        _CONSTS = _build_consts()
    beta = BETA_START + (BETA_END - BETA_START) * (
        t.astype(np.float64) / (NUM_TIMESTEPS - 1)
    )
    in_maps = []
    for c in range(NCORES):
        xc = x[c * NS : (c + 1) * NS].reshape(NS * M, D)
        yc = y[c * NS : (c + 1) * NS].reshape(NS * M, D)
        z = np.concatenate([xc, yc], axis=0)  # [64, D]
        # feature-major: zt[p, k*64 + r] = z[r, k*128 + p]
        zt = np.ascontiguousarray(
            z.reshape(R, NCH, 128).transpose(2, 1, 0).reshape(128, FREE)
        ).astype(NP_FP8)
        con = _CONSTS.copy()
        bcore = np.repeat(beta[c * NS : (c + 1) * NS], M)  # [32]
        con[0:32, _BV] = (-bcore / D).astype(np.float32)
        in_maps.append({"zt": zt, "con": con})
    return in_maps


def _run(x, y, t, trace=False, **spmd_kwargs):
    x = np.asarray(x, dtype=np.float32)
    y = np.asarray(y, dtype=np.float32)
    t = np.asarray(t, dtype=np.int32)
    nc = _get_prog()
    in_maps = _make_in_maps(x, y, t)
    br = run_bass_kernel_spmd(
        nc, in_maps, list(range(NCORES)), trace=trace, **spmd_kwargs
    )
    S = np.concatenate(
        [np.asarray(r["res"], dtype=np.float32) for r in br.results], axis=0
    )  # [32, 2] = per-sample [conf_sum, pairs + 2*conf_sum]
    conf = S[:, 0] / M
    pairs = S[:, 1] - 2.0 * S[:, 0]
    inter = pairs / (M * (M - 1))
    im = (LAMBDA_VAL / 2.0) * inter
    score = im - conf
    outs = tuple(
        np.ascontiguousarray(v, dtype=np.float32)
        for v in (score, conf, inter, im)
    )
    return outs, br


def kernel(x, y, t):
    """(score, confinement, interaction, interaction_mult), each [32] f32."""
    outs, _ = _run(x, y, t)
    return outs


# revision 23
# speedup vs baseline: 1.0815x; 1.0815x over previous
"""Trainium2 Bass kernel for nn_GeneralizedKernelScore (loss_fn).

Math per sample n (M=8 population members, D=12288 features):
    beta      = 2.0 - 1.9*t/999                      (linear schedule from t)
    conf[n]   = mean_j    exp(-beta*||x_j - y_j||^2 / D)
    inter[n]  = mean_{j!=j'} exp(-beta*||x_j - x_j'||^2 / D)
    im[n]     = inter/2
    score[n]  = im - conf

Strategy (data-parallel over batch, 4 samples per core on 8 cores):
Each core owns Z = [X; Y] (64 rows x 12288) in fp8-e4m3, pre-transposed
on the host to feature-major [128, 96*64] so the contraction dim lands
on SBUF partitions.  All distances come from the Gram matrix G = Z Z^T.
Feature chunks are processed two at a time: one matmul per pair with
lhsT = rhs = [chunk_j | chunk_j+1] ([128, 128]) accumulates
    P[0:64, 0:64]     += chunk_j   Gram contribution
    P[64:128, 64:128] += chunk_j+1 Gram contribution
(off-diagonal blocks are cross-chunk junk, ignored).  The 128-column
fp8 weight loads ride the fast-weight-load path and hide behind the
128-cycle streams; a short warm-up spin of junk matmuls starts the PE
early so the HAM clock gate reaches 2.4 GHz while the input still
streams in.

Epilogue (3 cross-engine hops):
  DVE   : xn2 = diag(G) via a stride-129 access pattern; one fused
          tensor_scalar builds the norm-routing rhs and the fold
          weights; one combined mask (same-sample block + x.y diag,
          disjoint) + grouped reduce compacts the -2G terms, with the
          x.y term landing in the f = p%8 slot
  PE    : three matmuls accumulate pt[32,8] = D*d2 args; the diag slot
          becomes the confinement arg, the [128->32] fold of the split
          Gram halves rides the contraction
  DVE   : extract the diag slot (conf arg) before the exp
  ACT   : two Exps with per-partition scale -beta/D (host-computed
          from t): pairs+conf row-sums via accum_out -> sc[:,0],
          conf -> sc[:,1]; the result DMA issues from this same
          engine's HWDGE queue (no extra hop)
  Host  : sums 8 rows per sample and applies the constant affine.

DMA: input split in 4 chunks (small first chunk for an early start)
issued alternately on the two HWDGE queues (SP + Activation) so the
rings drain in parallel; constants ride a 5th transfer.
"""

from contextlib import ExitStack

import numpy as np
import ml_dtypes

import concourse.bass as bass
from concourse.bass_types import AP
import concourse.mybir as mybir
import concourse.tile as tile
from concourse import bacc
from concourse.bass_utils import run_bass_kernel_spmd

# problem shape (hardcoded per spec)
N, M, D = 32, 8, 12288
NUM_TIMESTEPS = 1000
BETA_START, BETA_END = 2.0, 0.1
LAMBDA_VAL = 1.0

NCORES = 8
NS = N // NCORES          # 4 samples per core
R = 2 * NS * M            # 64 Z-rows per core (32 x-rows then 32 y-rows)
NCH = D // 128            # 96 contraction chunks of the feature dim
FREE = NCH * R            # 6144 free columns of Z^T
# input DMA chunk widths (bytes per partition line); must sum to FREE
# and stay multiples of 128 (one ldw-pair)
CHUNKS = [256, 1024, 2816, 2048]
N_WARM = 7                # PE warm-up matmuls (N=256 each, gapless to gram)
FILLERS = [2, 2, 0, 0]    # junk matmuls after each chunk's pairs: keep the
                          # PE busy through DMA gaps so HAM stays warming
DIAG_AP = False           # stride-129 diag AP (rejected by birverifier)

# const tensor column layout
_M2C, _I64, _MK8, _W3, _W2, _ON8, _MD, _BV, _P4 = (
    0, 128, 256, 264, 296, 328, 336, 344, 345,
)
CONW = 349

F32 = mybir.dt.float32
FP8 = mybir.dt.float8e4
NP_FP8 = ml_dtypes.float8_e4m3


def _build_consts():
    k = np.arange(128)[:, None]
    km = k % 64
    c = np.arange(128)[None, :]
    xrow = km < 32
    # combined -2 mask: same-sample x-x block (incl diag) + x.y diag;
    # disjoint regions, both land compatibly under the g=16 grouped sum
    m2c = np.where(
        (xrow & (c // 8 == k // 8) & (c % 64 < 32)) | (xrow & (c == k + 32)),
        -2.0, 0.0,
    )
    i64 = (c == k).astype(np.float32)  # fallback diag mask
    f8 = np.arange(8)[None, :]
    mk8 = (k % 8 == f8).astype(np.float32)       # norm routing by j = k%8
    m32 = np.arange(32)[None, :]
    w3 = (xrow & (km == m32)).astype(np.float32)  # fold [128]->[32], x-rows
    # W2 = A (same-sample x-rows) + B (own y-row) + C (own x-row);
    # arithmetic sum: A and C overlap on the own row, weight 2 there
    w2 = (
        (xrow & (km // 8 == m32 // 8)).astype(np.float32)
        + (km == 32 + m32).astype(np.float32)
        + (km == m32).astype(np.float32)
    )
    on8 = np.ones((128, 8), dtype=np.float32)
    md = (xrow & (k % 8 == f8)).astype(np.float32)[: 128]  # diag-slot mask
    bv = np.zeros((128, 1), dtype=np.float32)  # filled per-core with -beta/D
    p4 = ((k < 32) & (k // 8 == np.arange(4)[None, :])).astype(np.float32)
    con = np.concatenate(
        [m2c, i64, mk8, w3, w2, on8, md, bv, p4], axis=1
    ).astype(np.float32)
    assert con.shape == (128, CONW)
    return con


def _build_program():
    nc = bacc.Bacc("TRN2", target_bir_lowering=False)
    zt = nc.dram_tensor("zt", [128, FREE], FP8, kind="ExternalInput")
    con_d = nc.dram_tensor("con", [128, CONW], F32, kind="ExternalInput")
    res_d = nc.dram_tensor("res", [NS, 9], F32, kind="ExternalOutput")

    mult = mybir.AluOpType.mult
    EXP = mybir.ActivationFunctionType.Exp

    with ExitStack() as ctx:
        tc = ctx.enter_context(tile.TileContext(nc))
        small = ctx.enter_context(tc.tile_pool(name="small", bufs=1))
        zbf_p = ctx.enter_context(tc.tile_pool(name="zbf", bufs=len(CHUNKS)))
        psum = ctx.enter_context(tc.tile_pool(name="psum", bufs=1, space="PSUM"))

        # --- PE warm-up spin: open the HAM clock gate early -----------
        wt = small.tile([128, 256], FP8, tag="wt")
        nc.vector.memset(wt, 0.0)
        wp = psum.tile([128, 256], F32, tag="wp")
        for _ in range(N_WARM):
            nc.tensor.matmul(
                wp, lhsT=wt[:, 0:128], rhs=wt, start=True, stop=True,
                skip_group_check=True,
            )

        # --- input + const DMAs, alternating the two HWDGE queues -----
        zbf = []
        off = 0
        for i, w in enumerate(CHUNKS):
            zc = zbf_p.tile([128, w], FP8, tag="zbf")
            eng = nc.sync if i % 2 == 0 else nc.scalar
            eng.dma_start(out=zc, in_=zt[:, off : off + w])
            zbf.append(zc)
            off += w
        con = small.tile([128, CONW], F32, tag="con")
        nc.sync.dma_start(out=con, in_=con_d[:])

        # preload the Exp LUT while DMAs run
        warm = small.tile([1, 1], F32, tag="warm")
        nc.vector.memset(warm, 0.0)
        nc.scalar.activation(out=warm, in_=warm, func=EXP)

        # --- Gram: one [128,128] matmul per chunk pair ----------------
        G = psum.tile([128, 128], F32, tag="G")
        npair = NCH // 2
        p = 0
        for i, cw in enumerate(CHUNKS):
            for j in range(cw // 128):
                pair = zbf[i][:, j * 128 : (j + 1) * 128]
                nc.tensor.matmul(
                    G, lhsT=pair, rhs=pair,
                    start=(p == 0), stop=(p == npair - 1),
                )
                p += 1
            for _ in range(FILLERS[i]):
                nc.tensor.matmul(
                    wp, lhsT=wt[:, 0:128], rhs=wt, start=True, stop=True,
                    skip_group_check=True,
                )
        assert p == npair

        # --- epilogue ---------------------------------------------------
        # [V] xn2 = diag(G): split norms (even-chunk half on rows 0-63,
        # odd on 64-127)
        xn2 = small.tile([128, 1], F32, tag="xn2")
        if DIAG_AP:
            gdiag = AP(tensor=G.tensor, offset=G.offset, ap=[[129, 128], [1, 1]])
            nc.vector.tensor_copy(out=xn2, in_=gdiag)
        else:
            s128 = small.tile([128, 128], F32, tag="s128")
            nc.vector.tensor_tensor(
                out=s128, in0=G, in1=con[:, _I64 : _I64 + 128], op=mult
            )
            nc.vector.reduce_sum(out=xn2, in_=s128, axis=mybir.AxisListType.X)
        # rw = [mask8 | W3] . xn2 : rhs8 = rw[:,0:8], xnw = rw[:,8:40]
        rw = small.tile([128, 40], F32, tag="rw")
        nc.vector.tensor_scalar(
            out=rw, in0=con[:, _MK8 : _MK8 + 40], scalar1=xn2, scalar2=None,
            op0=mult,
        )
        gm = small.tile([128, 128], F32, tag="gm")
        nc.vector.tensor_tensor(
            out=gm, in0=G, in1=con[:, _M2C : _M2C + 128], op=mult
        )
        cmc = small.tile([128, 8], F32, tag="cmc")
        nc.vector.reduce_sum(
            out=cmc,
            in_=gm.rearrange("p (g f) -> p f g", g=16),
            axis=mybir.AxisListType.X,
        )

        # [T] pt[:,0:8] = norm-spread + row-norm bcast + (-2G, folded)
        pt = psum.tile([32, 9], F32, tag="pt")
        nc.tensor.matmul(
            pt[:, 0:8], lhsT=con[:, _W2 : _W2 + 32], rhs=rw[:, 0:8],
            start=True, stop=False,
        )
        nc.tensor.matmul(
            pt[:, 0:8], lhsT=rw[:, 8:40], rhs=con[:, _ON8 : _ON8 + 8],
            start=False, stop=False,
        )
        nc.tensor.matmul(
            pt[:, 0:8], lhsT=con[:, _W3 : _W3 + 32], rhs=cmc,
            start=False, stop=True,
        )

        # [V] copy the confinement arg (diag slot) into pt col 8
        md = small.tile([32, 8], F32, tag="md")
        nc.vector.tensor_tensor(
            out=md, in0=pt[:, 0:8], in1=con[0:32, _MD : _MD + 8], op=mult
        )
        nc.vector.reduce_sum(
            out=pt[:, 8:9], in_=md, axis=mybir.AxisListType.X
        )

        # [S] one exp over [32,9]: col 8 = conf, cols 0-7 pairs (diag
        # slot also conf)
        w = small.tile([32, 9], F32, tag="w")
        nc.scalar.activation(
            out=w, in_=pt, func=EXP, scale=con[0:32, _BV : _BV + 1]
        )

        # [T] per-sample sums over the 8 population rows (all 9 cols;
        # host folds the columns)
        pc = psum.tile([NS, 9], F32, tag="pc")
        nc.tensor.matmul(
            pc, lhsT=con[0:32, _P4 : _P4 + 4], rhs=w, start=True, stop=True
        )

        # [V] -> DMA out
        fin = small.tile([NS, 9], F32, tag="fin")
        nc.vector.tensor_copy(out=fin, in_=pc)
        nc.sync.dma_start(out=res_d[:], in_=fin)

    nc.compile()
    return nc


_PROG = None
_CONSTS = None


def _get_prog():
    global _PROG
    if _PROG is None:
        _PROG = _build_program()
    return _PROG


def _make_in_maps(x, y, t):
    global _CONSTS
    if _CONSTS is None:
        _CONSTS = _build_consts()
    beta = BETA_START + (BETA_END - BETA_START) * (
        t.astype(np.float64) / (NUM_TIMESTEPS - 1)
    )
    in_maps = []
    for c in range(NCORES):
        xc = x[c * NS : (c + 1) * NS].reshape(NS * M, D)
        yc = y[c * NS : (c + 1) * NS].reshape(NS * M, D)
        z = np.concatenate([xc, yc], axis=0)  # [64, D]
        # feature-major: zt[p, k*64 + r] = z[r, k*128 + p]
        zt = np.ascontiguousarray(
            z.reshape(R, NCH, 128).transpose(2, 1, 0).reshape(128, FREE)
        ).astype(NP_FP8)
        con = _CONSTS.copy()
        bcore = np.repeat(beta[c * NS : (c + 1) * NS], M)  # [32]
        con[0:32, _BV] = (-bcore / D).astype(np.float32)
        in_maps.append({"zt": zt, "con": con})
    return in_maps


def _run(x, y, t, trace=False, **spmd_kwargs):
    x = np.asarray(x, dtype=np.float32)
    y = np.asarray(y, dtype=np.float32)
    t = np.asarray(t, dtype=np.int32)
    nc = _get_prog()
    in_maps = _make_in_maps(x, y, t)
    br = run_bass_kernel_spmd(
        nc, in_maps, list(range(NCORES)), trace=trace, **spmd_kwargs
    )
    S = np.concatenate(
        [np.asarray(r["res"], dtype=np.float32) for r in br.results], axis=0
    )  # [32, 9]: per-sample sums of the 9 exp columns; col 8 = conf
    conf_sum = S[:, 8]
    pairs = S.sum(axis=1) - 2.0 * conf_sum
    conf = conf_sum / M
    inter = pairs / (M * (M - 1))
    im = (LAMBDA_VAL / 2.0) * inter
    score = im - conf
    outs = tuple(
        np.ascontiguousarray(v, dtype=np.float32)
        for v in (score, conf, inter, im)
    )
    return outs, br


def kernel(x, y, t):
    """(score, confinement, interaction, interaction_mult), each [32] f32."""
    outs, _ = _run(x, y, t)
    return outs
